# revision 11
# baseline (speedup 1.0000x reference)
"""Trainium2 Bass kernel for the B-spline (KAN-style) layer:

    out = einsum('bin,ion->bo', b_splines(tanh(x)), coeffs) + x @ base_weight

Key identity: with u = 4*tanh(x) + 7 in (3, 11) (uniform knots at integers
4..10 inside the range), each cubic B-spline basis value is b3(u - n), and the
space spanned by {b3(u-n)}_n over u in (3,11) is exactly {C^2 piecewise cubics
with knots 4..10} = span{1, w, w^2, w^3, relu(k-u)^3 (k=4,5,6), relu(u-k)^3
(k=7..10)} with w = u - 7. So the whole layer collapses to ONE matmul over
K = 11*1024 (x residual + 10 nonlinear planes per input feature; the constant
plane folds into a per-output bias applied at PSUM eviction) with
host-preconvolved weights.

Sharding: data-parallel over batch, 8 cores x 512 rows, weights replicated.
Matmul runs in float32r (~2^-12.5 effective operand rounding, full PE rate).
Plane-block order puts the x residual FIRST so the PE starts on raw DMA'd
x tiles with no elementwise work on the critical path.
"""
import numpy as np

import concourse.bass as bass
import concourse.mybir as mybir
import concourse.tile as tile
from concourse import bacc, bass_utils
from concourse.bass_interp import get_hw_module

B, F, O, NCTRL = 4096, 1024, 1024, 11
NCORES = 8
BS = B // NCORES          # 512 batch rows per core
P = 128
FT = F // P               # 8 feature tiles
OT = O // P               # 8 output tiles
NPLANES = 11              # residual + 10 nonlinear
KT = NPLANES * FT         # 88 k-tiles
F32 = mybir.dt.float32
F32R = mybir.dt.float32r
BF16 = mybir.dt.bfloat16
ACTF = mybir.ActivationFunctionType
ALU = mybir.AluOpType

# Planes whose folded weights stay f32r. The basis-collapse amplifies
# quantization noise ~16.6x (plane variances sum to 1459 vs output var 5.3 —
# huge cancellation). The top-5 variance planes (w2, w3, rho6, r7, r8 = 1434
# of 1459) keep f32r weights; bf16 on the rest adds only ~4e-3 maxrel and
# cuts their HBM traffic in half (46 MB -> 32 MB per core).
WEIGHT_F32_PLANES = (2, 3, 6, 7, 8)

# plane id -> kind: 0: x residual, 1: w=4t, 2: w^2, 3: w^3,
#                   4..6: relu(k-u)^3 k=4,5,6,  7..10: relu(u-k)^3 k=7..10
RHO_KNOTS = (4, 5, 6)
R_KNOTS = (7, 8, 9, 10)

_cached_program = None


def _build_program():
    nc = bacc.Bacc("TRN2", target_bir_lowering=False, debug=False,
                   enable_asserts=False, num_devices=NCORES)
    # const APs for float biases used by scalar.activation(Relu, bias=...)
    for v in (-1.0, -2.0, -3.0):
        ct = nc.alloc_sbuf_tensor(f"const-float32-{v}", [P, 1], F32)
        nc.gpsimd.memset(ct.ap(), v)
        nc.const_aps.aps[(F32, v)] = ct.ap()
    nc.all_engine_barrier()

    n32 = len(WEIGHT_F32_PLANES)
    n16 = NPLANES - n32
    xt_d = nc.dram_tensor("xt", [F, BS], F32R, kind="ExternalInput").ap()
    xb_d = nc.dram_tensor("xb", [F, BS], BF16, kind="ExternalInput").ap()
    wk32_d = nc.dram_tensor("wk32", [n32 * F, O], F32R, kind="ExternalInput").ap()
    wk16_d = nc.dram_tensor("wk16", [n16 * F, O], BF16, kind="ExternalInput").ap()
    bias_d = nc.dram_tensor("bias", [P, OT], F32, kind="ExternalInput").ap()
    out_d = nc.dram_tensor("out", [O, BS], F32, kind="ExternalOutput").ap()

    # plane index -> (dram ap, plane slot within that tensor, sbuf dtype)
    w_where = {}
    i32 = i16 = 0
    for p in range(NPLANES):
        if p in WEIGHT_F32_PLANES:
            w_where[p] = (wk32_d, i32, F32R)
            i32 += 1
        else:
            w_where[p] = (wk16_d, i16, BF16)
            i16 += 1

    with tile.TileContext(nc) as tc:
        with tc.tile_pool(name="const", bufs=1) as const_pool, \
             tc.tile_pool(name="tpool", bufs=1) as t_pool, \
             tc.tile_pool(name="qpool", bufs=3) as q_pool, \
             tc.tile_pool(name="ppool", bufs=4) as p_pool, \
             tc.tile_pool(name="wpool", bufs=8) as w_pool, \
             tc.tile_pool(name="epool", bufs=8) as e_pool, \
             tc.tile_pool(name="psum", bufs=1, space="PSUM") as psum_pool:

            # x tiles: f32r copy feeds tanh; bf16 copy feeds the residual
            # matmuls directly (matmul operands must share a dtype, and the
            # residual weights are bf16). gpsimd queue, so the sync queue
            # leads with the wk weight tiles.
            xts, xbs = [], []
            for f in range(FT):
                xt = t_pool.tile([P, BS], F32R, tag=f"xt{f}", name=f"xt{f}")
                nc.gpsimd.dma_start(xt[:], xt_d[f * P:(f + 1) * P, :])
                xts.append(xt)
                xb = t_pool.tile([P, BS], BF16, tag=f"xb{f}", name=f"xb{f}")
                nc.gpsimd.dma_start(xb[:], xb_d[f * P:(f + 1) * P, :])
                xbs.append(xb)

            bias_t = const_pool.tile([P, OT], F32)
            nc.gpsimd.dma_start(bias_t[:], bias_d)

            psums = [psum_pool.tile([P, BS], F32, tag=f"ps{o}", name=f"ps{o}")
                     for o in range(OT)]

            # HAM warmup: keep the PE busy while the first weight tiles DMA in,
            # so the real matmul stream starts at the warm clock. Writes into
            # psums are discarded by kt=0's start=True.
            warm_f = const_pool.tile([P, BS], F32)
            nc.vector.memset(warm_f[:], 0.0)
            warm = const_pool.tile([P, BS], F32R)
            nc.vector.tensor_copy(warm[:], warm_f[:])
            for i in range(8):
                nc.tensor.matmul(psums[i % OT][:], warm[:, 0:P], warm[:],
                                 start=True, stop=True, skip_group_check=True)

            # t = tanh(x) per feature tile (kept resident)
            ts_ = []
            for f in range(FT):
                tt = t_pool.tile([P, BS], F32, tag=f"t{f}", name=f"t{f}")
                nc.scalar.activation(tt[:], xts[f][:].bitcast(F32), ACTF.Tanh)
                ts_.append(tt)

            def make_plane(p, f):
                """Emit ops producing plane (p, f) as a [P, BS] tile whose
                dtype matches that plane's weight dtype."""
                if p == 0:          # residual: raw x tile, no compute
                    return xbs[f]
                pdt = F32R if p in WEIGHT_F32_PLANES else BF16
                t = ts_[f]
                pl = p_pool.tile([P, BS], pdt, tag=f"plane{pdt}", name=f"pl{p}_{f}")
                if p == 1:          # w = 4t
                    nc.scalar.activation(pl[:], t[:], ACTF.Copy, scale=4.0)
                elif p == 2:        # w^2 = (4t)^2
                    nc.scalar.activation(pl[:], t[:], ACTF.Square, scale=4.0)
                elif p == 3:        # w^3 = (64*t^2)*t
                    t2 = q_pool.tile([P, BS], F32, tag="q2", name=f"t2_{f}")
                    nc.scalar.activation(t2[:], t[:], ACTF.Square)
                    nc.vector.scalar_tensor_tensor(pl[:], t2[:], 64.0, t[:],
                                                   ALU.mult, ALU.mult)
                else:
                    if p <= 6:      # relu(k-u)^3 = relu(-4t + (k-7))^3
                        k = RHO_KNOTS[p - 4]
                        sc, bi = -4.0, float(k - 7)
                    else:           # relu(u-k)^3 = relu(4t + (7-k))^3
                        k = R_KNOTS[p - 7]
                        sc, bi = 4.0, float(7 - k)
                    q = q_pool.tile([P, BS], F32, tag="q", name=f"q{p}_{f}")
                    nc.scalar.activation(q[:], t[:], ACTF.Relu, scale=sc, bias=bi)
                    q2 = q_pool.tile([P, BS], F32, tag="q2", name=f"q2_{p}_{f}")
                    nc.scalar.activation(q2[:], q[:], ACTF.Square)
                    nc.vector.tensor_mul(pl[:], q2[:], q[:])
                return pl

            for kt in range(KT):
                p, f = divmod(kt, FT)
                pl = make_plane(p, f)
                wd, slot, wdt = w_where[p]
                row0 = (slot * FT + f) * P
                wt = w_pool.tile([P, O], wdt, tag=f"wk{wdt}", name=f"wk{kt}")
                nc.sync.dma_start(wt[:], wd[row0:row0 + P, :])
                for o in range(OT):
                    nc.tensor.matmul(psums[o][:], wt[:, o * P:(o + 1) * P], pl[:],
                                     start=(kt == 0), stop=(kt == KT - 1))

            # evict: out[o] = psum[o] + bias[:, o], split across Scalar/Vector,
            # out-DMAs split across sync/gpsimd queues
            for o in range(OT):
                ot = e_pool.tile([P, BS], F32, tag=f"evict{o % 2}", name=f"ev{o}")
                if o % 2 == 0:
                    nc.scalar.activation(ot[:], psums[o][:], ACTF.Identity,
                                         bias=bias_t[:, o:o + 1])
                else:
                    nc.vector.tensor_scalar_add(ot[:], psums[o][:],
                                                bias_t[:, o:o + 1])
                eng = (nc.sync, nc.gpsimd, nc.scalar)[o % 3]
                eng.dma_start(out_d[o * P:(o + 1) * P, :], ot[:])

    nc.compile()
    nc.m = get_hw_module(nc.m)
    return nc


def _precompute_weights(coeffs, base_weight):
    """Fold the B-spline basis change into the coefficient tensor.

    b3(v) = (1/6) sum_{j=0..4} C4[j] relu(v-j)^3,  C4 = (1,-4,6,-4,1)
    activation = sum_n coeffs[:,:,n] b3(u-n) = sum_j beta_j relu(u-j)^3
    with u in (3,11):
      j<=3   -> (u-j)^3 exactly        -> monomials in w = u-7 (+ constant)
      4..6   -> (u-j)^3 + relu(j-u)^3  -> monomials + rho_j
      7..10  -> relu(u-j)^3            -> r_j
      j>=11  -> 0
    Returns wk [11*F, O] float32 (plane-block order: residual, w, w^2, w^3,
    rho4..6, r7..10) and bias [P, OT] float32.
    """
    F_, O_, N_ = coeffs.shape
    c = coeffs.astype(np.float64)
    C4 = np.array([1.0, -4.0, 6.0, -4.0, 1.0]) / 6.0
    beta = np.zeros((F_, O_, 15))
    for n in range(N_):
        for j in range(5):
            beta[:, :, n + j] += c[:, :, n] * C4[j]

    const_w = np.zeros((F_, O_))
    mono_w = np.zeros((F_, O_, 3))    # w, w^2, w^3
    rho_w = np.zeros((F_, O_, 3))     # knots 4,5,6 reflected
    r_w = np.zeros((F_, O_, 4))       # knots 7..10
    for j in range(11):
        a = 7.0 - j                   # (u-j)^3 = (w+a)^3
        if j <= 6:
            const_w += beta[:, :, j] * a ** 3
            mono_w[:, :, 0] += beta[:, :, j] * 3 * a ** 2
            mono_w[:, :, 1] += beta[:, :, j] * 3 * a
            mono_w[:, :, 2] += beta[:, :, j]
            if j >= 4:
                rho_w[:, :, j - 4] += beta[:, :, j]
        else:
            r_w[:, :, j - 7] += beta[:, :, j]

    wk = np.concatenate([
        base_weight.astype(np.float64).reshape(F_, O_),
        mono_w.transpose(2, 0, 1).reshape(3 * F_, O_),
        rho_w.transpose(2, 0, 1).reshape(3 * F_, O_),
        r_w.transpose(2, 0, 1).reshape(4 * F_, O_),
    ], axis=0).astype(np.float32)
    bias = const_w.sum(axis=0)                         # [O]
    bias2d = bias.reshape(OT, P).T.astype(np.float32)  # [P, OT], o = j*128 + p
    return np.ascontiguousarray(wk), np.ascontiguousarray(bias2d)


def _split_weights(wk):
    """Split [11F, O] fp32 weights into the f32r and bf16 plane stacks."""
    import ml_dtypes
    wk3 = wk.reshape(NPLANES, F, O)
    w32 = np.ascontiguousarray(
        wk3[list(WEIGHT_F32_PLANES)].reshape(-1, O).astype(np.float32))
    other = [p for p in range(NPLANES) if p not in WEIGHT_F32_PLANES]
    w16 = np.ascontiguousarray(
        wk3[other].reshape(-1, O).astype(ml_dtypes.bfloat16))
    return w32, w16


def _core_inputs(x, coeffs, base_weight, core):
    import ml_dtypes
    wk, bias2d = _precompute_weights(coeffs, base_weight)
    w32, w16 = _split_weights(wk)
    xs = np.ascontiguousarray(x[core * BS:(core + 1) * BS, :].T)  # [F, BS]
    xb = np.ascontiguousarray(xs.astype(ml_dtypes.bfloat16))
    return {"xt": xs, "xb": xb, "wk32": w32, "wk16": w16, "bias": bias2d}


def _assemble_output(outs, cores):
    out = np.empty((len(cores) * BS, O), np.float32)
    for i, c in enumerate(cores):
        out[i * BS:(i + 1) * BS, :] = outs[i].T
    return out


def kernel(x, coeffs, base_weight, grid):
    global _cached_program
    x = np.asarray(x, np.float32)
    coeffs = np.asarray(coeffs, np.float32)
    base_weight = np.asarray(base_weight, np.float32)

    wk, bias2d = _precompute_weights(coeffs, base_weight)
    w32, w16 = _split_weights(wk)
    if _cached_program is None:
        _cached_program = _build_program()
    nc = _cached_program

    import ml_dtypes
    in_maps = []
    for c in range(NCORES):
        xs = np.ascontiguousarray(x[c * BS:(c + 1) * BS, :].T)  # [F, BS]
        xb = np.ascontiguousarray(xs.astype(ml_dtypes.bfloat16))
        in_maps.append({"xt": xs, "xb": xb, "wk32": w32, "wk16": w16,
                        "bias": bias2d})

    res = bass_utils.run_bass_kernel_spmd(nc, in_maps, core_ids=list(range(NCORES)))
    out = np.empty((B, O), np.float32)
    for c in range(NCORES):
        out[c * BS:(c + 1) * BS, :] = res.results[c]["out"].T
    return out



# revision 33
# speedup vs baseline: 1671.5791x; 1671.5791x over previous
"""Trainium2 Bass kernel for the B-spline (KAN-style) layer:

    out = einsum('bin,ion->bo', b_splines(tanh(x)), coeffs) + x @ base_weight

Key identity: with u = 4*tanh(x) + 7 in (3, 11) (uniform knots at integers
4..10 inside the range), each cubic B-spline basis value is b3(u - n), and the
space spanned by {b3(u-n)}_n over u in (3,11) is exactly {C^2 piecewise cubics
with knots 4..10} = span{1, w, w^2, w^3, relu(k-u)^3 (k=4,5,6), relu(u-k)^3
(k=7..10)} with w = u - 7. So the whole layer collapses to ONE matmul over
K = 11*1024 (x residual + 10 nonlinear planes per input feature; the constant
plane folds into a per-output bias applied at PSUM eviction) with
host-preconvolved weights.

Sharding: data-parallel over batch, 8 cores x 512 rows, weights replicated.

Precision: the basis collapse amplifies operand-quantization noise ~16.6x
(plane variances sum to 1459 vs output variance 5.3 - massive cancellation
between planes), which rules out bf16/fp8 for the high-variance planes.
fp16 (e5m10) keeps maxrel ~8e-3 (< 2e-2 budget) at full PE rate with 2-byte
weights: half the HBM traffic of f32r and cheap (overlappable) stationary
loads. The two negligible-variance planes (rho4, r10) run as one fp8-e4m3
DoubleRow pair per feature tile - two k-tiles per PE pass.

Plane-block order puts the x residual FIRST so the PE starts on raw DMA'd
x tiles with no elementwise work on the critical path.
"""
import numpy as np

import concourse.bass as bass
import concourse.mybir as mybir
import concourse.tile as tile
from concourse import bacc, bass_utils
from concourse.bass_interp import get_hw_module

B, F, O, NCTRL = 4096, 1024, 1024, 11
NCORES = 8
BS = B // NCORES          # 512 batch rows per core
P = 128
FT = F // P               # 8 feature tiles
OT = O // P               # 8 output tiles
NPLANES = 11              # residual + 10 nonlinear
KT = NPLANES * FT         # 88 k-tiles
F32 = mybir.dt.float32
F32R = mybir.dt.float32r
BF16 = mybir.dt.bfloat16
F16 = mybir.dt.float16
FP8 = mybir.dt.float8e4
ACTF = mybir.ActivationFunctionType
ALU = mybir.AluOpType

# Planes whose folded weights stay f32r (none: fp16 everywhere passes the
# error budget with 2.4x margin and halves weight HBM traffic vs f32r; bf16
# would NOT pass - only 7 mantissa bits against the 16.6x noise amplification).
WEIGHT_F32_PLANES = ()
# The two lowest-variance planes (0.04 each of 1459) run as ONE fp8-e4m3
# DoubleRow pair per f-tile: 2 k-tiles per PE pass instead of 1. Weights
# scaled x2, planes x0.5 (product unscaled; shared PSUM). Adds ~2e-3 maxrel.
FP8_PAIR = (4, 10)
FP8_WSCALE = 2.0

# Bench-only experiment knobs (wrong results; timing isolation):
BENCH_RESIDENT_W = False   # reuse one weight tile per dtype (no per-kt DMA)
BENCH_RESIDENT_P = False   # reuse one plane tile per dtype (no elementwise)
BENCH_DMA_ONLY = False     # emit only the DMA streams, no compute


def _plane_groups():
    f32p = [p for p in range(NPLANES)
            if p in WEIGHT_F32_PLANES and p not in FP8_PAIR]
    bf16p = [p for p in range(NPLANES)
             if p not in WEIGHT_F32_PLANES and p not in FP8_PAIR]
    return f32p, bf16p

# plane id -> kind: 0: x residual, 1: w=4t, 2: w^2, 3: w^3,
#                   4..6: relu(k-u)^3 k=4,5,6,  7..10: relu(u-k)^3 k=7..10
RHO_KNOTS = (4, 5, 6)
R_KNOTS = (7, 8, 9, 10)

_cached_program = None


def _build_program(bench_iters=0):
    """Build the SPMD program. bench_iters>0 wraps the whole body in a
    hardware For_i loop (benchmark-only variant for A/B timing through the
    axon tunnel, where per-dispatch overhead is ~300ms)."""
    nc = bacc.Bacc("TRN2", target_bir_lowering=False, debug=False,
                   enable_asserts=False, num_devices=NCORES)
    # const APs for float biases used by scalar.activation(Relu, bias=...)
    for v in (-1.0, -2.0, -3.0):
        ct = nc.alloc_sbuf_tensor(f"const-float32-{v}", [P, 1], F32)
        nc.gpsimd.memset(ct.ap(), v)
        nc.const_aps.aps[(F32, v)] = ct.ap()
    nc.all_engine_barrier()

    f32p, bf16p = _plane_groups()
    n32 = max(1, len(f32p))
    n16 = max(1, len(bf16p))
    xt_d = nc.dram_tensor("xt", [F, BS], F32R, kind="ExternalInput").ap()
    xb_d = nc.dram_tensor("xb", [F, BS], F16, kind="ExternalInput").ap()
    wk32_d = nc.dram_tensor("wk32", [n32 * F, O], F32R, kind="ExternalInput").ap()
    wk16_d = nc.dram_tensor("wk16", [n16 * F, O], F16, kind="ExternalInput").ap()
    wk8_d = nc.dram_tensor("wk8", [F, 2 * O], FP8, kind="ExternalInput").ap()
    bias_d = nc.dram_tensor("bias", [P, OT], F32, kind="ExternalInput").ap()
    out_d = nc.dram_tensor("out", [O, BS], F32, kind="ExternalOutput").ap()

    # plane index -> (dram ap, plane slot within that tensor, sbuf dtype)
    w_where = {p: (wk32_d, i, F32R) for i, p in enumerate(f32p)}
    w_where.update({p: (wk16_d, i, F16) for i, p in enumerate(bf16p)})
    norm_planes = [p for p in range(NPLANES) if p not in FP8_PAIR]

    with tile.TileContext(nc) as tc:
        with tc.tile_pool(name="const", bufs=1) as const_pool, \
             tc.tile_pool(name="tpool", bufs=1) as t_pool, \
             tc.tile_pool(name="qpool", bufs=6) as q_pool, \
             tc.tile_pool(name="ppool", bufs=6) as p_pool, \
             tc.tile_pool(name="p8pool", bufs=2) as p8_pool, \
             tc.tile_pool(name="wpool", bufs=12) as w_pool, \
             tc.tile_pool(name="w8pool", bufs=2) as w8_pool, \
             tc.tile_pool(name="epool", bufs=8) as e_pool, \
             tc.tile_pool(name="psum", bufs=1, space="PSUM") as psum_pool:

            def body():
                # x tiles: f32r copy feeds tanh; fp16 copy feeds the residual
                # matmuls directly (matmul operands must share a dtype, and
                # the residual weights are fp16). gpsimd queue, so the sync
                # queue leads with the wk weight tiles.
                xts, xbs = [], []
                for f in range(FT):
                    xt = t_pool.tile([P, BS], F32R, tag=f"xt{f}", name=f"xt{f}")
                    nc.gpsimd.dma_start(xt[:], xt_d[f * P:(f + 1) * P, :])
                    xts.append(xt)
                    xb = t_pool.tile([P, BS], F16, tag=f"xb{f}", name=f"xb{f}")
                    nc.gpsimd.dma_start(xb[:], xb_d[f * P:(f + 1) * P, :])
                    xbs.append(xb)

                bias_t = const_pool.tile([P, OT], F32)
                nc.gpsimd.dma_start(bias_t[:], bias_d)

                psums = [psum_pool.tile([P, BS], F32, tag=f"ps{o}", name=f"ps{o}")
                         for o in range(OT)]

                # HAM warmup: keep the PE busy while the first weight tiles
                # DMA in, so the real matmul stream starts at the warm clock.
                # Writes into psums are discarded by kt=0's start=True.
                warm_f = const_pool.tile([P, BS], F32)
                nc.vector.memset(warm_f[:], 0.0)
                warm = const_pool.tile([P, BS], F32R)
                nc.vector.tensor_copy(warm[:], warm_f[:])
                for i in range(8):
                    nc.tensor.matmul(psums[i % OT][:], warm[:, 0:P], warm[:],
                                     start=True, stop=True, skip_group_check=True)

                # t = tanh(x) per feature tile (kept resident)
                ts_ = []
                for f in range(FT):
                    tt = t_pool.tile([P, BS], F32, tag=f"t{f}", name=f"t{f}")
                    nc.scalar.activation(tt[:], xts[f][:].bitcast(F32), ACTF.Tanh)
                    ts_.append(tt)

                def relu_q(p, f, out, out_scale):
                    """out = out_scale * relu(sc*t + bi)^3 for relu plane p."""
                    t = ts_[f]
                    if p <= 6:      # relu(k-u)^3 = relu(-4t + (k-7))^3
                        k = RHO_KNOTS[p - 4]
                        sc, bi = -4.0, float(k - 7)
                    else:           # relu(u-k)^3 = relu(4t + (7-k))^3
                        k = R_KNOTS[p - 7]
                        sc, bi = 4.0, float(7 - k)
                    q = q_pool.tile([P, BS], F32, tag="q", name=f"q{p}_{f}")
                    nc.scalar.activation(q[:], t[:], ACTF.Relu, scale=sc, bias=bi)
                    q2 = q_pool.tile([P, BS], F32, tag="q2", name=f"q2_{p}_{f}")
                    nc.scalar.activation(q2[:], q[:], ACTF.Square)
                    if out_scale == 1.0:
                        nc.vector.tensor_mul(out, q2[:], q[:])
                    else:
                        nc.vector.scalar_tensor_tensor(out, q2[:], out_scale,
                                                       q[:], ALU.mult, ALU.mult)

                def make_plane(p, f):
                    """Emit ops producing plane (p, f) as a [P, BS] tile whose
                    dtype matches that plane's weight dtype."""
                    if p == 0:          # residual: raw x tile, no compute
                        return xbs[f]
                    pdt = F32R if p in WEIGHT_F32_PLANES else F16
                    t = ts_[f]
                    pl = p_pool.tile([P, BS], pdt, tag=f"plane{pdt}",
                                     name=f"pl{p}_{f}")
                    if p == 1:          # w = 4t
                        nc.scalar.activation(pl[:], t[:], ACTF.Copy, scale=4.0)
                    elif p == 2:        # w^2 = (4t)^2
                        nc.scalar.activation(pl[:], t[:], ACTF.Square, scale=4.0)
                    elif p == 3:        # w^3 = (64*t^2)*t
                        t2 = q_pool.tile([P, BS], F32, tag="q2", name=f"t2_{f}")
                        nc.scalar.activation(t2[:], t[:], ACTF.Square)
                        nc.vector.scalar_tensor_tensor(pl[:], t2[:], 64.0, t[:],
                                                       ALU.mult, ALU.mult)
                    else:
                        relu_q(p, f, pl[:], 1.0)
                    return pl

                resident_w = {}
                if BENCH_RESIDENT_W:
                    for dt_, dram in ((F32R, wk32_d), (F16, wk16_d)):
                        rt = const_pool.tile([P, O], dt_, tag=f"resw{dt_}")
                        nc.sync.dma_start(rt[:], dram[0:P, :])
                        resident_w[dt_] = rt
                resident_p = {}
                if BENCH_RESIDENT_P:
                    for dt_ in (F32R, F16):
                        rp = const_pool.tile([P, BS], dt_, tag=f"resp{dt_}")
                        nc.vector.tensor_copy(rp[:], warm_f[:])
                        resident_p[dt_] = rp
                    rp8 = const_pool.tile([P, 2, BS], FP8, tag="resp8")
                    nc.vector.tensor_copy(rp8[:, 0, :], warm_f[:])
                    nc.vector.tensor_copy(rp8[:, 1, :], warm_f[:])
                    resident_p[FP8] = rp8

                nkt = len(norm_planes) * FT
                for kt in range(nkt):
                    pi, f = divmod(kt, FT)
                    p = norm_planes[pi]
                    wd, slot, wdt = w_where[p]
                    if BENCH_DMA_ONLY:
                        wt = w_pool.tile([P, O], wdt, tag=f"wk{wdt}")
                        row0 = (slot * FT + f) * P
                        nc.sync.dma_start(wt[:], wd[row0:row0 + P, :])
                        continue
                    if BENCH_RESIDENT_P:
                        pl = resident_p[F32R if p in WEIGHT_F32_PLANES else F16]
                    else:
                        pl = make_plane(p, f)
                    if BENCH_RESIDENT_W:
                        wt = resident_w[wdt]
                    else:
                        row0 = (slot * FT + f) * P
                        wt = w_pool.tile([P, O], wdt, tag=f"wk{wdt}",
                                         name=f"wk{kt}")
                        nc.sync.dma_start(wt[:], wd[row0:row0 + P, :])
                    for o in range(OT):
                        nc.tensor.matmul(psums[o][:],
                                         wt[:, o * P:(o + 1) * P], pl[:],
                                         start=(kt == 0),
                                         stop=(not FP8_PAIR and kt == nkt - 1))

                # fp8 DoubleRow pairs: planes FP8_PAIR packed two-k-tiles-per
                # pass. Weights pre-scaled x FP8_WSCALE on host; planes
                # divided by it here, so PSUM contributions are unscaled.
                for f in range(FT if (FP8_PAIR and not BENCH_DMA_ONLY) else 0):
                    if BENCH_RESIDENT_P:
                        pp = resident_p[FP8]
                    else:
                        pp = p8_pool.tile([P, 2, BS], FP8, tag="p8",
                                          name=f"p8_{f}")
                        for s, p in enumerate(FP8_PAIR):
                            relu_q(p, f, pp[:, s, :], 1.0 / FP8_WSCALE)
                    wp = w8_pool.tile([P, 2, O], FP8, tag="w8", name=f"w8_{f}")
                    nc.sync.dma_start(wp[:], wk8_d[f * P:(f + 1) * P, :])
                    for o in range(OT):
                        nc.tensor.matmul(psums[o][:],
                                         wp[:, :, o * P:(o + 1) * P], pp[:],
                                         start=False, stop=(f == FT - 1),
                                         perf_mode=mybir.MatmulPerfMode.DoubleRow)
                if BENCH_DMA_ONLY:
                    return

                # evict: out[o] = psum[o] + bias[:, o], split across
                # Scalar/Vector, out-DMAs split across sync/gpsimd queues
                for o in range(OT):
                    ot = e_pool.tile([P, BS], F32, tag=f"evict{o % 2}",
                                     name=f"ev{o}")
                    if o % 2 == 0:
                        nc.scalar.activation(ot[:], psums[o][:], ACTF.Identity,
                                             bias=bias_t[:, o:o + 1])
                    else:
                        nc.vector.tensor_scalar_add(ot[:], psums[o][:],
                                                    bias_t[:, o:o + 1])
                    eng = (nc.sync, nc.gpsimd, nc.scalar)[o % 3]
                    eng.dma_start(out_d[o * P:(o + 1) * P, :], ot[:])

            if bench_iters:
                with tc.For_i(0, bench_iters, 1,
                              hint_engines=(mybir.EngineType.PE,)):
                    body()
            else:
                body()

    nc.compile()
    nc.m = get_hw_module(nc.m)
    return nc


def _precompute_weights(coeffs, base_weight):
    """Fold the B-spline basis change into the coefficient tensor.

    b3(v) = (1/6) sum_{j=0..4} C4[j] relu(v-j)^3,  C4 = (1,-4,6,-4,1)
    activation = sum_n coeffs[:,:,n] b3(u-n) = sum_j beta_j relu(u-j)^3
    with u in (3,11):
      j<=3   -> (u-j)^3 exactly        -> monomials in w = u-7 (+ constant)
      4..6   -> (u-j)^3 + relu(j-u)^3  -> monomials + rho_j
      7..10  -> relu(u-j)^3            -> r_j
      j>=11  -> 0
    Returns wk [11*F, O] float32 (plane-block order: residual, w, w^2, w^3,
    rho4..6, r7..10) and bias [P, OT] float32.
    """
    F_, O_, N_ = coeffs.shape
    c = coeffs.astype(np.float64)
    C4 = np.array([1.0, -4.0, 6.0, -4.0, 1.0]) / 6.0
    beta = np.zeros((F_, O_, 15))
    for n in range(N_):
        for j in range(5):
            beta[:, :, n + j] += c[:, :, n] * C4[j]

    const_w = np.zeros((F_, O_))
    mono_w = np.zeros((F_, O_, 3))    # w, w^2, w^3
    rho_w = np.zeros((F_, O_, 3))     # knots 4,5,6 reflected
    r_w = np.zeros((F_, O_, 4))       # knots 7..10
    for j in range(11):
        a = 7.0 - j                   # (u-j)^3 = (w+a)^3
        if j <= 6:
            const_w += beta[:, :, j] * a ** 3
            mono_w[:, :, 0] += beta[:, :, j] * 3 * a ** 2
            mono_w[:, :, 1] += beta[:, :, j] * 3 * a
            mono_w[:, :, 2] += beta[:, :, j]
            if j >= 4:
                rho_w[:, :, j - 4] += beta[:, :, j]
        else:
            r_w[:, :, j - 7] += beta[:, :, j]

    wk = np.concatenate([
        base_weight.astype(np.float64).reshape(F_, O_),
        mono_w.transpose(2, 0, 1).reshape(3 * F_, O_),
        rho_w.transpose(2, 0, 1).reshape(3 * F_, O_),
        r_w.transpose(2, 0, 1).reshape(4 * F_, O_),
    ], axis=0).astype(np.float32)
    bias = const_w.sum(axis=0)                         # [O]
    bias2d = bias.reshape(OT, P).T.astype(np.float32)  # [P, OT], o = j*128 + p
    return np.ascontiguousarray(wk), np.ascontiguousarray(bias2d)


def _split_weights(wk):
    """Split [11F, O] fp32 weights into f32r / bf16 / fp8-pair stacks."""
    import ml_dtypes
    wk3 = wk.reshape(NPLANES, F, O)
    f32p, bf16p = _plane_groups()

    def stack(planes, dt):
        if not planes:
            return np.zeros((F, O), dt)
        return np.ascontiguousarray(wk3[planes].reshape(-1, O).astype(dt))

    w32 = stack(f32p, np.float32)
    w16 = stack(bf16p, np.float16)
    # [F, 2*O]: row f*128+p = [plane FP8_PAIR[0] | plane FP8_PAIR[1]] * scale
    if FP8_PAIR:
        w8 = np.stack([wk3[p] * FP8_WSCALE for p in FP8_PAIR], axis=1)
        w8 = w8.reshape(F, 2 * O)
    else:
        w8 = np.zeros((F, 2 * O), np.float32)
    w8 = np.ascontiguousarray(w8.astype(ml_dtypes.float8_e4m3))
    return w32, w16, w8


def _core_inputs(x, coeffs, base_weight, core):
    import ml_dtypes
    wk, bias2d = _precompute_weights(coeffs, base_weight)
    w32, w16, w8 = _split_weights(wk)
    xs = np.ascontiguousarray(x[core * BS:(core + 1) * BS, :].T)  # [F, BS]
    xb = np.ascontiguousarray(xs.astype(np.float16))
    return {"xt": xs, "xb": xb, "wk32": w32, "wk16": w16, "wk8": w8,
            "bias": bias2d}


def _assemble_output(outs, cores):
    out = np.empty((len(cores) * BS, O), np.float32)
    for i, c in enumerate(cores):
        out[i * BS:(i + 1) * BS, :] = outs[i].T
    return out


def kernel(x, coeffs, base_weight, grid):
    global _cached_program
    x = np.asarray(x, np.float32)
    coeffs = np.asarray(coeffs, np.float32)
    base_weight = np.asarray(base_weight, np.float32)

    if _cached_program is None:
        _cached_program = _build_program()
    nc = _cached_program

    in_maps = [_core_inputs(x, coeffs, base_weight, c) for c in range(NCORES)]

    res = bass_utils.run_bass_kernel_spmd(nc, in_maps, core_ids=list(range(NCORES)))
    out = np.empty((B, O), np.float32)
    for c in range(NCORES):
        out[c * BS:(c + 1) * BS, :] = res.results[c]["out"].T
    return out



# revision 53
# speedup vs baseline: 1787.1044x; 1.0691x over previous
"""Trainium2 Bass kernel for the B-spline (KAN-style) layer:

    out = einsum('bin,ion->bo', b_splines(tanh(x)), coeffs) + x @ base_weight

Key identity: with u = 4*tanh(x) + 7 in (3, 11) (uniform knots at integers
4..10 inside the range), each cubic B-spline basis value is b3(u - n), and the
space spanned by {b3(u-n)}_n over u in (3,11) is exactly {C^2 piecewise cubics
with knots 4..10} = span{1, w, w^2, w^3, relu(k-u)^3 (k=4,5,6), relu(u-k)^3
(k=7..10)} with w = u - 7. So the whole layer collapses to ONE matmul over
K = 11*1024 (x residual + 10 nonlinear planes per input feature; the constant
plane folds into a per-output bias applied at PSUM eviction) with
host-preconvolved weights.

Sharding: data-parallel over batch, 8 cores x 512 rows, weights replicated.

Precision: the basis collapse amplifies operand-quantization noise ~16.6x
(plane variances sum to 1459 vs output variance 5.3 - massive cancellation
between planes), which rules out bf16/fp8 for the high-variance planes.
fp16 (e5m10) keeps maxrel ~8e-3 (< 2e-2 budget) at full PE rate with 2-byte
weights: half the HBM traffic of f32r and cheap (overlappable) stationary
loads. The two negligible-variance planes (rho4, r10) run as one fp8-e4m3
DoubleRow pair per feature tile - two k-tiles per PE pass.

Plane-block order puts the x residual FIRST so the PE starts on raw DMA'd
x tiles with no elementwise work on the critical path.
"""
import numpy as np

import concourse.bass as bass
import concourse.mybir as mybir
import concourse.tile as tile
from concourse import bacc, bass_utils
from concourse.bass_interp import get_hw_module

B, F, O, NCTRL = 4096, 1024, 1024, 11
NCORES = 8
BS = B // NCORES          # 512 batch rows per core
P = 128
FT = F // P               # 8 feature tiles
OT = O // P               # 8 output tiles
NPLANES = 11              # residual + 10 nonlinear
KT = NPLANES * FT         # 88 k-tiles
F32 = mybir.dt.float32
F32R = mybir.dt.float32r
BF16 = mybir.dt.bfloat16
F16 = mybir.dt.float16
FP8 = mybir.dt.float8e4
ACTF = mybir.ActivationFunctionType
ALU = mybir.AluOpType

# Planes whose folded weights stay f32r (none: fp16 everywhere passes the
# error budget with 2.4x margin and halves weight HBM traffic vs f32r; bf16
# would NOT pass - only 7 mantissa bits against the 16.6x noise amplification).
WEIGHT_F32_PLANES = ()
# The two lowest-variance planes (0.04 each of 1459) run as ONE fp8-e4m3
# DoubleRow pair per f-tile: 2 k-tiles per PE pass instead of 1. Weights
# scaled x2, planes x0.5 (product unscaled; shared PSUM). Adds ~2e-3 maxrel.
FP8_PAIR = (4, 10)
FP8_WSCALE = 2.0

# Bench-only experiment knobs (wrong results; timing isolation):
BENCH_RESIDENT_W = False   # reuse one weight tile per dtype (no per-kt DMA)
BENCH_RESIDENT_P = False   # reuse one plane tile per dtype (no elementwise)
BENCH_DMA_ONLY = False     # emit only the DMA streams, no compute
BENCH_SHARED_LDW = 1       # stationary slices per ktile: o-slice = o//this*this

# Swapped arrangement: planes are the stationary operand (shared by the two
# o-half matmuls -> half the distinct weight loads), weights are the moving
# operand, PSUM holds [batch-chunk, o-half] so the output lands as [BS, O].
SWAP = False

# Pair-pack the fp16 planes (all but the x residual): two k-tiles share one
# plane tile and one weight DMA, halving PE semaphore traffic and weight-DMA
# issue count. Measured WORSE than per-k-tile tiles in an interleaved A/B
# (+2.5us min / +12.5us med per iteration) - the dependencies were already
# pre-satisfied and the coarser tiles hurt producer pipelining. Keep off.
PAIR_PACK = False


def _plane_groups():
    f32p = [p for p in range(NPLANES)
            if p in WEIGHT_F32_PLANES and p not in FP8_PAIR]
    bf16p = [p for p in range(NPLANES)
             if p not in WEIGHT_F32_PLANES and p not in FP8_PAIR]
    return f32p, bf16p

# plane id -> kind: 0: x residual, 1: w=4t, 2: w^2, 3: w^3,
#                   4..6: relu(k-u)^3 k=4,5,6,  7..10: relu(u-k)^3 k=7..10
RHO_KNOTS = (4, 5, 6)
R_KNOTS = (7, 8, 9, 10)

_cached_program = None


def _build_program(bench_iters=0):
    """Build the SPMD program. bench_iters>0 wraps the whole body in a
    hardware For_i loop (benchmark-only variant for A/B timing through the
    axon tunnel, where per-dispatch overhead is ~300ms)."""
    nc = bacc.Bacc("TRN2", target_bir_lowering=False, debug=False,
                   enable_asserts=False, num_devices=NCORES)
    # const APs for float biases used by scalar.activation(Relu, bias=...)
    for v in (-1.0, -2.0, -3.0):
        ct = nc.alloc_sbuf_tensor(f"const-float32-{v}", [P, 1], F32)
        nc.gpsimd.memset(ct.ap(), v)
        nc.const_aps.aps[(F32, v)] = ct.ap()
    nc.all_engine_barrier()

    f32p, bf16p = _plane_groups()
    n32 = max(1, len(f32p))
    n16 = max(1, len(bf16p))
    xt_d = nc.dram_tensor("xt", [F, BS], F32R, kind="ExternalInput").ap()
    xb_d = nc.dram_tensor("xb", [F, BS], F16, kind="ExternalInput").ap()
    wk32_d = nc.dram_tensor("wk32", [n32 * F, O], F32R, kind="ExternalInput").ap()
    wk16_d = nc.dram_tensor("wk16", [n16 * F, O], F16, kind="ExternalInput").ap()
    if PAIR_PACK:
        wk16p_d = nc.dram_tensor("wk16p", [max(1, len(bf16p) - 1) * F, O], F16,
                                 kind="ExternalInput").ap()
    wk8_d = nc.dram_tensor("wk8", [F, 2 * O], FP8, kind="ExternalInput").ap()
    bias_d = nc.dram_tensor("bias", [P, OT], F32, kind="ExternalInput").ap()
    if SWAP:
        biasb_d = nc.dram_tensor("biasb", [P, O], F32, kind="ExternalInput").ap()
        out_d = nc.dram_tensor("out", [BS, O], F32, kind="ExternalOutput").ap()
    else:
        out_d = nc.dram_tensor("out", [O, BS], F32, kind="ExternalOutput").ap()

    # plane index -> (dram ap, plane slot within that tensor, sbuf dtype)
    w_where = {p: (wk32_d, i, F32R) for i, p in enumerate(f32p)}
    w_where.update({p: (wk16_d, i, F16) for i, p in enumerate(bf16p)})
    norm_planes = [p for p in range(NPLANES) if p not in FP8_PAIR]

    with tile.TileContext(nc) as tc:
        with tc.tile_pool(name="const", bufs=1) as const_pool, \
             tc.tile_pool(name="tpool", bufs=1) as t_pool, \
             tc.tile_pool(name="qpool", bufs=6) as q_pool, \
             tc.tile_pool(name="ppool", bufs=6) as p_pool, \
             tc.tile_pool(name="p8pool", bufs=2) as p8_pool, \
             tc.tile_pool(name="wpool", bufs=12) as w_pool, \
             tc.tile_pool(name="w8pool", bufs=2) as w8_pool, \
             tc.tile_pool(name="epool", bufs=8) as e_pool, \
             tc.tile_pool(name="psum", bufs=1, space="PSUM") as psum_pool:

            def body():
                # x tiles: f32r copy feeds tanh; fp16 copy feeds the residual
                # matmuls directly (matmul operands must share a dtype, and
                # the residual weights are fp16). gpsimd queue, so the sync
                # queue leads with the wk weight tiles.
                xts, xbs = [], []
                for f in range(FT):
                    xt = t_pool.tile([P, BS], F32R, tag=f"xt{f}", name=f"xt{f}")
                    nc.gpsimd.dma_start(xt[:], xt_d[f * P:(f + 1) * P, :])
                    xts.append(xt)
                    xb = t_pool.tile([P, BS], F16, tag=f"xb{f}", name=f"xb{f}")
                    nc.gpsimd.dma_start(xb[:], xb_d[f * P:(f + 1) * P, :])
                    xbs.append(xb)

                if SWAP:
                    biasb_t = const_pool.tile([P, O], F32)
                    nc.gpsimd.dma_start(biasb_t[:], biasb_d)
                else:
                    bias_t = const_pool.tile([P, OT], F32)
                    nc.gpsimd.dma_start(bias_t[:], bias_d)

                psums = [psum_pool.tile([P, BS], F32, tag=f"ps{o}", name=f"ps{o}")
                         for o in range(OT)]

                # HAM warmup: keep the PE busy while the first weight tiles
                # DMA in, so the real matmul stream starts at the warm clock.
                # Writes into psums are discarded by kt=0's start=True.
                warm_f = const_pool.tile([P, BS], F32)
                nc.vector.memset(warm_f[:], 0.0)
                warm = const_pool.tile([P, BS], F32R)
                nc.vector.tensor_copy(warm[:], warm_f[:])
                for i in range(8):
                    nc.tensor.matmul(psums[i % OT][:], warm[:, 0:P], warm[:],
                                     start=True, stop=True, skip_group_check=True)

                # t = tanh(x) per feature tile (kept resident)
                ts_ = []
                for f in range(FT):
                    tt = t_pool.tile([P, BS], F32, tag=f"t{f}", name=f"t{f}")
                    nc.scalar.activation(tt[:], xts[f][:].bitcast(F32), ACTF.Tanh)
                    ts_.append(tt)

                def relu_q(p, f, out, out_scale):
                    """out = out_scale * relu(sc*t + bi)^3 for relu plane p."""
                    t = ts_[f]
                    if p <= 6:      # relu(k-u)^3 = relu(-4t + (k-7))^3
                        k = RHO_KNOTS[p - 4]
                        sc, bi = -4.0, float(k - 7)
                    else:           # relu(u-k)^3 = relu(4t + (7-k))^3
                        k = R_KNOTS[p - 7]
                        sc, bi = 4.0, float(7 - k)
                    q = q_pool.tile([P, BS], F32, tag="q", name=f"q{p}_{f}")
                    nc.scalar.activation(q[:], t[:], ACTF.Relu, scale=sc, bias=bi)
                    q2 = q_pool.tile([P, BS], F32, tag="q2", name=f"q2_{p}_{f}")
                    nc.scalar.activation(q2[:], q[:], ACTF.Square)
                    if out_scale == 1.0:
                        nc.vector.tensor_mul(out, q2[:], q[:])
                    else:
                        nc.vector.scalar_tensor_tensor(out, q2[:], out_scale,
                                                       q[:], ALU.mult, ALU.mult)

                def make_plane_into(p, f, out):
                    """Emit ops producing plane (p, f) into the AP `out`."""
                    t = ts_[f]
                    if p == 1:          # w = 4t
                        nc.scalar.activation(out, t[:], ACTF.Copy, scale=4.0)
                    elif p == 2:        # w^2 = (4t)^2
                        nc.scalar.activation(out, t[:], ACTF.Square, scale=4.0)
                    elif p == 3:        # w^3 = (64*t^2)*t
                        t2 = q_pool.tile([P, BS], F32, tag="q2", name=f"t2_{f}")
                        nc.scalar.activation(t2[:], t[:], ACTF.Square)
                        nc.vector.scalar_tensor_tensor(out, t2[:], 64.0, t[:],
                                                       ALU.mult, ALU.mult)
                    else:
                        relu_q(p, f, out, 1.0)

                def make_plane(p, f):
                    """Emit ops producing plane (p, f) as a [P, BS] tile whose
                    dtype matches that plane's weight dtype."""
                    if p == 0:          # residual: raw x tile, no compute
                        return xbs[f]
                    pdt = F32R if p in WEIGHT_F32_PLANES else F16
                    pl = p_pool.tile([P, BS], pdt, tag=f"plane{pdt}",
                                     name=f"pl{p}_{f}")
                    make_plane_into(p, f, pl[:])
                    return pl

                resident_w = {}
                if BENCH_RESIDENT_W:
                    for dt_, dram in ((F32R, wk32_d), (F16, wk16_d)):
                        rt = const_pool.tile([P, O], dt_, tag=f"resw{dt_}")
                        nc.sync.dma_start(rt[:], dram[0:P, :])
                        resident_w[dt_] = rt
                resident_p = {}
                if BENCH_RESIDENT_P:
                    for dt_ in (F32R, F16):
                        rp = const_pool.tile([P, BS], dt_, tag=f"resp{dt_}")
                        nc.vector.tensor_copy(rp[:], warm_f[:])
                        resident_p[dt_] = rp
                    rp8 = const_pool.tile([P, 2, BS], FP8, tag="resp8")
                    nc.vector.tensor_copy(rp8[:, 0, :], warm_f[:])
                    nc.vector.tensor_copy(rp8[:, 1, :], warm_f[:])
                    resident_p[FP8] = rp8

                pair_mode = (PAIR_PACK and not SWAP and not BENCH_DMA_ONLY
                             and not BENCH_RESIDENT_W and not BENCH_RESIDENT_P)
                if pair_mode:
                    # plane 0 (x residual): 8 unpaired k-tiles off raw DMA
                    xslot = bf16p.index(0)
                    for f in range(FT):
                        wt = w_pool.tile([P, O], F16, tag="wkx", name=f"wkx{f}")
                        row0 = (xslot * FT + f) * P
                        nc.sync.dma_start(wt[:], wk16_d[row0:row0 + P, :])
                        for o in range(OT):
                            nc.tensor.matmul(psums[o][:],
                                             wt[:, o * P:(o + 1) * P],
                                             xbs[f][:], start=(f == 0),
                                             stop=False)
                    # remaining fp16 planes: two k-tiles per plane/weight tile
                    paired = [p for p in bf16p if p != 0]
                    for q, p in enumerate(paired):
                        for g in range(FT // 2):
                            pp2 = p_pool.tile([P, 2, BS], F16, tag="ppair",
                                              name=f"pp{p}_{g}")
                            for s in range(2):
                                make_plane_into(p, 2 * g + s, pp2[:, s, :])
                            wp2 = w_pool.tile([P, 2, O], F16, tag="wpair",
                                              name=f"wp{p}_{g}")
                            r0 = (q * (FT // 2) + g) * 2 * P
                            nc.sync.dma_start(wp2[:], wk16p_d[r0:r0 + 2 * P, :])
                            for s in range(2):
                                for o in range(OT):
                                    nc.tensor.matmul(
                                        psums[o][:],
                                        wp2[:, s, o * P:(o + 1) * P],
                                        pp2[:, s, :], start=False, stop=False)

                nkt = 0 if pair_mode else len(norm_planes) * FT
                for kt in range(nkt):
                    pi, f = divmod(kt, FT)
                    p = norm_planes[pi]
                    wd, slot, wdt = w_where[p]
                    if BENCH_DMA_ONLY:
                        wt = w_pool.tile([P, O], wdt, tag=f"wk{wdt}")
                        row0 = (slot * FT + f) * P
                        nc.sync.dma_start(wt[:], wd[row0:row0 + P, :])
                        continue
                    if BENCH_RESIDENT_P:
                        pl = resident_p[F32R if p in WEIGHT_F32_PLANES else F16]
                    else:
                        pl = make_plane(p, f)
                    if BENCH_RESIDENT_W:
                        wt = resident_w[wdt]
                    else:
                        row0 = (slot * FT + f) * P
                        wt = w_pool.tile([P, O], wdt, tag=f"wk{wdt}",
                                         name=f"wk{kt}")
                        nc.sync.dma_start(wt[:], wd[row0:row0 + P, :])
                    last = (not FP8_PAIR and kt == nkt - 1)
                    if SWAP:
                        for j in range(4):
                            lhsT = pl[:, j * P:(j + 1) * P]
                            for h in range(2):
                                nc.tensor.matmul(
                                    psums[j * 2 + h][:], lhsT,
                                    wt[:, h * 512:(h + 1) * 512],
                                    start=(kt == 0), stop=last)
                    else:
                        for o in range(OT):
                            os_ = (o // BENCH_SHARED_LDW) * BENCH_SHARED_LDW
                            nc.tensor.matmul(psums[o][:],
                                             wt[:, os_ * P:(os_ + 1) * P], pl[:],
                                             start=(kt == 0), stop=last)

                # fp8 DoubleRow pairs: planes FP8_PAIR packed two-k-tiles-per
                # pass. Weights pre-scaled x FP8_WSCALE on host; planes
                # divided by it here, so PSUM contributions are unscaled.
                for f in range(FT if (FP8_PAIR and not BENCH_DMA_ONLY) else 0):
                    if BENCH_RESIDENT_P:
                        pp = resident_p[FP8]
                    else:
                        pp = p8_pool.tile([P, 2, BS], FP8, tag="p8",
                                          name=f"p8_{f}")
                        for s, p in enumerate(FP8_PAIR):
                            relu_q(p, f, pp[:, s, :], 1.0 / FP8_WSCALE)
                    wp = w8_pool.tile([P, 2, O], FP8, tag="w8", name=f"w8_{f}")
                    nc.sync.dma_start(wp[:], wk8_d[f * P:(f + 1) * P, :])
                    if SWAP:
                        for j in range(4):
                            lhsT = pp[:, :, j * P:(j + 1) * P]
                            for h in range(2):
                                nc.tensor.matmul(
                                    psums[j * 2 + h][:], lhsT,
                                    wp[:, :, h * 512:(h + 1) * 512],
                                    start=False, stop=(f == FT - 1),
                                    perf_mode=mybir.MatmulPerfMode.DoubleRow)
                    else:
                        for o in range(OT):
                            nc.tensor.matmul(
                                psums[o][:], wp[:, :, o * P:(o + 1) * P], pp[:],
                                start=False, stop=(f == FT - 1),
                                perf_mode=mybir.MatmulPerfMode.DoubleRow)
                if BENCH_DMA_ONLY:
                    return

                if SWAP:
                    # evict: out[j*128+, h*512+] = psum[j,h] + bias (bias
                    # varies along the FREE dim -> tensor_add on DVE)
                    for i in range(OT):
                        j, h = divmod(i, 2)
                        ot = e_pool.tile([P, 512], F32, tag=f"evict{i % 2}",
                                         name=f"ev{i}")
                        nc.vector.tensor_add(ot[:], psums[i][:],
                                             biasb_t[:, h * 512:(h + 1) * 512])
                        eng = (nc.sync, nc.gpsimd, nc.scalar)[i % 3]
                        eng.dma_start(
                            out_d[j * P:(j + 1) * P, h * 512:(h + 1) * 512],
                            ot[:])
                else:
                    # evict: out[o] = psum[o] + bias[:, o], split across
                    # Scalar/Vector, out-DMAs split across sync/gpsimd queues
                    for o in range(OT):
                        ot = e_pool.tile([P, BS], F32, tag=f"evict{o % 2}",
                                         name=f"ev{o}")
                        if o % 2 == 0:
                            nc.scalar.activation(ot[:], psums[o][:],
                                                 ACTF.Identity,
                                                 bias=bias_t[:, o:o + 1])
                        else:
                            nc.vector.tensor_scalar_add(ot[:], psums[o][:],
                                                        bias_t[:, o:o + 1])
                        eng = (nc.sync, nc.gpsimd, nc.scalar)[o % 3]
                        eng.dma_start(out_d[o * P:(o + 1) * P, :], ot[:])

            if bench_iters:
                with tc.For_i(0, bench_iters, 1,
                              hint_engines=(mybir.EngineType.PE,)):
                    body()
            else:
                body()

    nc.compile()
    nc.m = get_hw_module(nc.m)
    return nc


def _precompute_weights(coeffs, base_weight):
    """Fold the B-spline basis change into the coefficient tensor.

    b3(v) = (1/6) sum_{j=0..4} C4[j] relu(v-j)^3,  C4 = (1,-4,6,-4,1)
    activation = sum_n coeffs[:,:,n] b3(u-n) = sum_j beta_j relu(u-j)^3
    with u in (3,11):
      j<=3   -> (u-j)^3 exactly        -> monomials in w = u-7 (+ constant)
      4..6   -> (u-j)^3 + relu(j-u)^3  -> monomials + rho_j
      7..10  -> relu(u-j)^3            -> r_j
      j>=11  -> 0
    Returns wk [11*F, O] float32 (plane-block order: residual, w, w^2, w^3,
    rho4..6, r7..10) and bias [P, OT] float32.
    """
    F_, O_, N_ = coeffs.shape
    c = coeffs.astype(np.float64)
    C4 = np.array([1.0, -4.0, 6.0, -4.0, 1.0]) / 6.0
    beta = np.zeros((F_, O_, 15))
    for n in range(N_):
        for j in range(5):
            beta[:, :, n + j] += c[:, :, n] * C4[j]

    const_w = np.zeros((F_, O_))
    mono_w = np.zeros((F_, O_, 3))    # w, w^2, w^3
    rho_w = np.zeros((F_, O_, 3))     # knots 4,5,6 reflected
    r_w = np.zeros((F_, O_, 4))       # knots 7..10
    for j in range(11):
        a = 7.0 - j                   # (u-j)^3 = (w+a)^3
        if j <= 6:
            const_w += beta[:, :, j] * a ** 3
            mono_w[:, :, 0] += beta[:, :, j] * 3 * a ** 2
            mono_w[:, :, 1] += beta[:, :, j] * 3 * a
            mono_w[:, :, 2] += beta[:, :, j]
            if j >= 4:
                rho_w[:, :, j - 4] += beta[:, :, j]
        else:
            r_w[:, :, j - 7] += beta[:, :, j]

    wk = np.concatenate([
        base_weight.astype(np.float64).reshape(F_, O_),
        mono_w.transpose(2, 0, 1).reshape(3 * F_, O_),
        rho_w.transpose(2, 0, 1).reshape(3 * F_, O_),
        r_w.transpose(2, 0, 1).reshape(4 * F_, O_),
    ], axis=0).astype(np.float32)
    bias = const_w.sum(axis=0)                         # [O]
    bias2d = bias.reshape(OT, P).T.astype(np.float32)  # [P, OT], o = j*128 + p
    return np.ascontiguousarray(wk), np.ascontiguousarray(bias2d)


def _split_weights(wk):
    """Split [11F, O] fp32 weights into f32r / bf16 / fp8-pair stacks."""
    import ml_dtypes
    wk3 = wk.reshape(NPLANES, F, O)
    f32p, bf16p = _plane_groups()

    def stack(planes, dt):
        if not planes:
            return np.zeros((F, O), dt)
        return np.ascontiguousarray(wk3[planes].reshape(-1, O).astype(dt))

    w32 = stack(f32p, np.float32)
    w16 = stack(bf16p, np.float16)
    # pair-packed copy of the non-x fp16 planes: each 256-row (2 k-tile)
    # block row-permuted so a single strided DMA lands slot-major in SBUF:
    # src_block[2p+s] = block[s*128+p]
    w16p = None
    if PAIR_PACK:
        paired = [p for p in bf16p if p != 0]
        if paired:
            blocks = []
            for p in paired:
                for g in range(FT // 2):
                    blk = wk3[p][g * 256:(g + 1) * 256]
                    blocks.append(blk.reshape(2, P, O).transpose(1, 0, 2)
                                  .reshape(2 * P, O))
            w16p = np.ascontiguousarray(
                np.concatenate(blocks, axis=0).astype(np.float16))
        else:
            w16p = np.zeros((F, O), np.float16)
    # [F, 2*O]: row f*128+p = [plane FP8_PAIR[0] | plane FP8_PAIR[1]] * scale
    if FP8_PAIR:
        w8 = np.stack([wk3[p] * FP8_WSCALE for p in FP8_PAIR], axis=1)
        w8 = w8.reshape(F, 2 * O)
    else:
        w8 = np.zeros((F, 2 * O), np.float32)
    w8 = np.ascontiguousarray(w8.astype(ml_dtypes.float8_e4m3))
    return w32, w16, w8, w16p


def _core_inputs(x, coeffs, base_weight, core):
    import ml_dtypes
    wk, bias2d = _precompute_weights(coeffs, base_weight)
    w32, w16, w8, w16p = _split_weights(wk)
    xs = np.ascontiguousarray(x[core * BS:(core + 1) * BS, :].T)  # [F, BS]
    xb = np.ascontiguousarray(xs.astype(np.float16))
    m = {"xt": xs, "xb": xb, "wk32": w32, "wk16": w16, "wk8": w8,
         "bias": bias2d}
    if PAIR_PACK:
        m["wk16p"] = w16p
    if SWAP:
        bias_vec = np.ascontiguousarray(bias2d.T).reshape(O)  # bias[j*128+p]
        m["biasb"] = np.ascontiguousarray(
            np.broadcast_to(bias_vec[None, :], (P, O)).astype(np.float32))
    return m


def _assemble_output(outs, cores):
    out = np.empty((len(cores) * BS, O), np.float32)
    for i, c in enumerate(cores):
        out[i * BS:(i + 1) * BS, :] = outs[i] if SWAP else outs[i].T
    return out


def kernel(x, coeffs, base_weight, grid):
    global _cached_program
    x = np.asarray(x, np.float32)
    coeffs = np.asarray(coeffs, np.float32)
    base_weight = np.asarray(base_weight, np.float32)

    if _cached_program is None:
        _cached_program = _build_program()
    nc = _cached_program

    in_maps = [_core_inputs(x, coeffs, base_weight, c) for c in range(NCORES)]

    res = bass_utils.run_bass_kernel_spmd(nc, in_maps, core_ids=list(range(NCORES)))
    out = np.empty((B, O), np.float32)
    for c in range(NCORES):
        o = res.results[c]["out"]
        out[c * BS:(c + 1) * BS, :] = o if SWAP else o.T
    return out

